# revision 120
# baseline (speedup 1.0000x reference)
"""Trainium2 Bass kernel for nn_Attention_42288247996512 (sparse causal cross-attention).

reference:
  q = x @ Wq.T; k = cross @ Wk.T; v = x @ Wv.T
  logits = q @ k.T  (causal mask; padding mask m_q*m_k + eye > 0)
  out = softmax(logits / sqrt(128)) @ v

Key observation: with the padding mask + eye, a masked query's attention row
collapses to the diagonal -> out[q] = v[q] exactly, and masked keys drop out
of every unmasked query's softmax. Compacting each batch to its unmasked
positions (sorted) turns the problem into a PURE causal attention of length
n ~= 1024 (half of S) with no padding mask at all; masked rows are served
straight from the V projection.

Sharding: 8 cores = 4 batches x 2 parities. Keys are ranked unmasked-first;
core p owns ranks == p (mod 2) -> exactly 1024 V rows per core (no waste),
and the 2:1 rank interleave gives both parities identical causal widths
(true SPMD). Queries are replicated (q-range is the matmul free axis);
AV partial sums + denominators are added on the host.

On-chip, logits are built TRANSPOSED ([key, query]) so exp's per-partition
bias kills invalid tail keys, the causal boundary is one 256-wide 0/1
multiply after exp, and AV consumes e directly (no PE transposes). The
denominator rides the AV accumulation as a ones-vector matmul. All matmul
operands are bf16 (psum f32): full PE rate at any free width, half DMA.
"""
import math
import threading

import ml_dtypes
import numpy as np

B, S, D, DA = 4, 2048, 1024, 128
P = 128
NCORES = 8
KC = D // P    # 8 contraction chunks
NQ = 1056      # padded compact query length (max n = 1052)
NKL = 576      # local attention key slots per core (4.5 chunks, max na=526)
NV = 1024      # v keys per core (8 chunks)
NCH = 5        # local attention chunks
BIG = 32768.0

_BUILD_LOCK = threading.Lock()
_CACHE: dict = {}


def _build():
    from contextlib import ExitStack

    import concourse.bass as bass
    import concourse.mybir as mybir
    import concourse.tile as tile
    from concourse import bacc

    dt = mybir.dt
    f32 = dt.float32
    bf16 = dt.bfloat16
    AF = mybir.ActivationFunctionType
    ALU = mybir.AluOpType

    nc = bacc.Bacc("TRN2", target_bir_lowering=False, debug=False)

    # xv and wv are host-packed into one tensor, column order
    # [xv 0:128 | wv dm 0:512 | xv 128:1024 | wv dm 512:1024], so the first
    # matmul's operands are ONE contiguous 640-col DMA piece and each later
    # kc chunk is a single DMA: one HWDGE gen + one 900ns sem, not two.
    xw = nc.dram_tensor("xw", [D, 2 * NV], bf16, kind="ExternalInput").ap()
    xq = nc.dram_tensor("xq", [D, NQ], bf16, kind="ExternalInput").ap()
    ck = nc.dram_tensor("ck", [D, NKL], bf16, kind="ExternalInput").ap()
    # wq/wk arrive host-packed as [P, KC*DA] so one DMA with 2KB rows
    # loads them (a [D, DA] layout would need 256B descriptors: 2x penalty)
    wq = nc.dram_tensor("wq", [P, KC * DA], bf16, kind="ExternalInput").ap()
    wk = nc.dram_tensor("wk", [P, KC * DA], bf16, kind="ExternalInput").ap()
    kbias = nc.dram_tensor("kbias", [P, NCH], f32, kind="ExternalInput").ap()
    tri = nc.dram_tensor("tri", [P, 256], bf16, kind="ExternalInput").ap()
    ones = nc.dram_tensor("ones", [P, 1], bf16, kind="ExternalInput").ap()

    avT = nc.dram_tensor("avT", [D, NQ], bf16, kind="ExternalOutput").ap()
    den = nc.dram_tensor("den", [1, NQ], f32, kind="ExternalOutput").ap()
    vout = nc.dram_tensor("vout", [NV - 384, D], bf16,
                          kind="ExternalOutput").ap()

    xw_r = xw.rearrange("(kc p) k -> p kc k", p=P)

    with tile.TileContext(nc) as tc, ExitStack() as ctx:
        const = ctx.enter_context(tc.tile_pool(name="const", bufs=1))
        persist = ctx.enter_context(tc.tile_pool(name="persist", bufs=1))

        # persistent SBUF tensors
        xw_sb = persist.tile([P, KC, 2 * NV], bf16, name="xw_sb")
        ck_sb = persist.tile([P, KC, NKL], bf16, name="ck_sb")
        xq_sb = persist.tile([P, KC, NQ], bf16, name="xq_sb")
        wq_sb = const.tile([P, KC, DA], bf16, name="wq_sb")
        wk_sb = const.tile([P, KC, DA], bf16, name="wk_sb")
        kbias_sb = const.tile([P, NCH], f32, name="kbias_sb")
        tri_sb = const.tile([P, 256], bf16, name="tri_sb")
        ones_sb = const.tile([P, 1], bf16, name="ones_sb")

        v_sb = persist.tile([P, KC, D], bf16, name="v_sb")
        kT_sb = persist.tile([P, NKL], bf16, name="kT_sb")
        qT_sb = persist.tile([P, NQ], bf16, name="qT_sb")
        e_sb = [persist.tile([64 if c == NCH - 1 else P, (9 - 2 * c) * P],
                             bf16, name=f"e{c}")
                for c in range(NCH)]
        av_sb = persist.tile([P, KC, NQ], bf16, name="av_sb")
        den_sb = persist.tile([1, NQ], f32, name="den_sb")

        # explicit copy-engine pinning: the any-scheduler leaves Pool idle
        # and lets blocked waits on one SEQ gate unrelated later work
        def dve_cp(o, i):
            nc.vector.tensor_copy(o, i)

        def act_cp(o, i):
            nc.scalar.copy(o, i)

        # ---- input DMA, in consumption order ----
        # kc0 is split fine across two queues (HWDGE gens serialize in
        # arrival order, so the ACT-queued weight piece generates first and
        # the first matmul's operands land ~1us earlier); kc 1-7 are one
        # packed xv|wv DMA each.
        nc.sync.dma_start(xw_sb[:, 0, 0:768], xw_r[:, 0, 0:768])
        nc.sync.dma_start(xw_sb[:, 0, 768:1536], xw_r[:, 0, 768:1536])
        for kc in range(1, KC):
            nc.sync.dma_start(xw_sb[:, kc, :], xw_r[:, kc, :])
        # kc0's wv half-1 columns aren't consumed until V half 1 (~20us in):
        # deferring them pulls kc1's arrival a full queue slot earlier
        nc.sync.dma_start(xw_sb[:, 0, 1536:2048], xw_r[:, 0, 1536:2048])
        nc.sync.dma_start(ck_sb[:], ck.rearrange("(kc p) k -> p kc k", p=P))
        nc.sync.dma_start(wk_sb[:], wk.rearrange("p (kc m) -> p kc m", kc=KC))
        nc.sync.dma_start(wq_sb[:], wq.rearrange("p (kc m) -> p kc m", kc=KC))
        nc.sync.dma_start(xq_sb[:], xq.rearrange("(kc p) k -> p kc k", p=P))
        nc.sync.dma_start(kbias_sb[:], kbias[:])
        nc.sync.dma_start(tri_sb[:], tri[:])
        nc.sync.dma_start(ones_sb[:], ones[:])

        # One PSUM pool for the whole kernel, banks addressed by tag so
        # cross-phase reuse is per-bank WAR (no pool-boundary barrier).
        with tc.tile_pool(name="ps8", bufs=1, space="PSUM") as ps8:
            def bank(i, name, shape=None):
                return ps8.tile(shape or [P, 512], f32, tag=f"b{i}",
                                name=name)

            # ---- phase V: v = xv.T @ wv, 8 key chunks x 1024 dm ----
            # half 0: kc-outer (streams with the DMA FIFO) on all 8 banks;
            # half 1: j-outer on banks 2-7 with inline copies, so banks 0/1
            # are long free for K and each bank's WAR cleared 6 chains back.
            def xv_sl(kc, j):  # xv j-block in the permuted xw layout
                return xw_sb[:, kc, 0:128] if j == 0 \
                    else xw_sb[:, kc, 512 + j * P:512 + (j + 1) * P]

            ps = [bank(j, f"pv0_{j}") for j in range(8)]
            for kc in range(KC):
                for j in range(8):
                    nc.tensor.matmul(
                        ps[j][:],
                        lhsT=xv_sl(kc, j),
                        rhs=xw_sb[:, kc, 128:640],
                        start=(kc == 0), stop=(kc == KC - 1),
                    )
            for j in range(8):
                (dve_cp if j % 2 else act_cp)(v_sb[:, j, 0:512], ps[j][:])
            def k_proj():
                k_ps = [bank(5, "psk0"), bank(6, "psk1")]
                for kc in range(KC):
                    nc.tensor.matmul(k_ps[0][:], lhsT=wk_sb[:, kc, :],
                                     rhs=ck_sb[:, kc, 0:512],
                                     start=(kc == 0), stop=(kc == KC - 1))
                    nc.tensor.matmul(k_ps[1][:, 0:64], lhsT=wk_sb[:, kc, :],
                                     rhs=ck_sb[:, kc, 512:576],
                                     start=(kc == 0), stop=(kc == KC - 1))
                # split so logits chunk 0 (lhsT = kT[:, 0:128]) unblocks early
                dve_cp(kT_sb[:, 0:128], k_ps[0][:, 0:128])
                act_cp(kT_sb[:, 128:512], k_ps[0][:, 128:512])
                dve_cp(kT_sb[:, 512:576], k_ps[1][:, 0:64])

            for j in range(8):
                psj = bank(j % 3, f"pv1_{j}")
                for kc in range(KC):
                    nc.tensor.matmul(
                        psj[:],
                        lhsT=xv_sl(kc, j),
                        rhs=xw_sb[:, kc, 1536:2048],
                        start=(kc == 0), stop=(kc == KC - 1),
                    )
                (dve_cp if j % 2 else act_cp)(v_sb[:, j, 512:1024], psj[:])
                if j == 5:
                    # K projection rides inside half 1 on banks 5/6 (idle
                    # since half 0) — absorbs the V->K transition gap
                    k_proj()

            # masked-key v rows (local i >= 384) stream while attention runs
            nc.sync.dma_start(
                vout.rearrange("(c p) m -> p c m", p=P), v_sb[:, 3:8, :])

            # ---- K / Q projections fused with the first logits pieces ----
            # Emission order pipelines phase boundaries: each projection's
            # psum->sbuf copy lands while the next chain runs on the PE, and
            # the first logits/exp pieces interleave with the Q tail so AV
            # for chunk 0 starts with e_0 already built.
            avT_r = avT.rearrange("(dmc p) q -> p dmc q", p=P)
            npsl = 0

            def kp(c):  # key partitions of chunk c (last chunk is 64-high)
                return min(P, NKL - c * P)

            def logits_piece(c, off, sz):
                # one <=512-wide logits+exp piece; causal tri rides after
                # piece 0 — AV's diagonal step only needs cols 0:256
                nonlocal npsl
                h = kp(c)
                psl = bank(2 + npsl % 2, f"psl{c}_{off}")
                npsl += 1
                nc.tensor.matmul(
                    psl[0:h, 0:sz],
                    lhsT=kT_sb[:, c * P:c * P + h],
                    rhs=qT_sb[:, c * 256 + off:c * 256 + off + sz],
                    start=True, stop=True)
                nc.scalar.activation(
                    e_sb[c][:, off:off + sz], psl[0:h, 0:sz], AF.Exp,
                    bias=kbias_sb[0:h, c:c + 1], scale=1.0)
                if off == 0:
                    bw = min(256, (9 - 2 * c) * P - (1152 - NQ))
                    nc.vector.tensor_tensor(
                        out=e_sb[c][:, 0:bw], in0=e_sb[c][:, 0:bw],
                        in1=tri_sb[0:h, 0:bw], op=ALU.mult)

            def pieces(c):
                w = (9 - 2 * c) * P - (1152 - NQ)
                off = 0
                while off < w:
                    yield off, min(512, w - off)
                    off += 512

            def logits_chunk(c):
                for off, sz in pieces(c):
                    logits_piece(c, off, sz)

            def q_chain(i):
                sz = NQ - 1024 if i == 2 else 512
                q_ps = bank((7 + i) % 8, f"psq{i}")
                for kc in range(KC):
                    nc.tensor.matmul(
                        q_ps[:, 0:sz], lhsT=wq_sb[:, kc, :],
                        rhs=xq_sb[:, kc, i * 512:i * 512 + sz],
                        start=(kc == 0), stop=(kc == KC - 1))
                dve_cp(qT_sb[:, i * 512:i * 512 + sz], q_ps[:, 0:sz])


            # ---- attention AV/den pipeline; AV chains on banks 5,6,7,0
            # (cleared >=1us earlier by K/Q copies), den on bank 1.
            # NOTE: GPSIMD/Pool cannot read PSUM (BIR verifier rule the
            # timeline sim does not model) — psum->sbuf copies go DVE/ACT.
            av_banks = [[4, 5, 6, 7], [0, 4, 5, 6]]
            av_cp = [[dve_cp, act_cp, dve_cp, act_cp],
                     [act_cp, dve_cp, act_cp, dve_cp]]

            def av_sub(c, hw_, bl, dmcs, cps, eng):
                # one sub-unit: AV chains for `dmcs` + copies + output DMA
                avp = [bank(bl[i], f"av{c}_{dmcs[i]}", [P, 256])
                       for i in range(len(dmcs))]
                for cc in range(c + 1):
                    for i, dmc in enumerate(dmcs):
                        nc.tensor.matmul(
                            avp[i][:, 0:hw_],
                            lhsT=v_sb[0:kp(cc), cc, dmc * P:(dmc + 1) * P],
                            rhs=e_sb[cc][:, (c - cc) * 256:
                                         (c - cc) * 256 + hw_],
                            start=(cc == 0), stop=(cc == c))
                for i, dmc in enumerate(dmcs):
                    cps[i](av_sb[:, dmc, c * 256:c * 256 + hw_],
                           avp[i][:, 0:hw_])
                eng.dma_start(
                    avT_r[:, dmcs[0]:dmcs[-1] + 1, c * 256:c * 256 + hw_],
                    av_sb[:, dmcs[0]:dmcs[-1] + 1, c * 256:c * 256 + hw_])

            def av_grp(c, grp, hw_, final=False):
                if grp == 1 and c >= NCH - 2:
                    bl = [0, 2, 3, 1]
                elif grp == 1 and c <= 1:
                    # chunk 0's group 1: banks whose writers (q1, den0, and
                    # group 0's two earliest copies) cleared longest ago
                    bl = [0, 1, 4, 5]
                else:
                    bl = av_banks[grp]
                if c >= NCH - 2:
                    cps = [act_cp, dve_cp, act_cp, dve_cp]
                else:
                    cps = av_cp[grp]
                d0 = grp * 4
                if final:
                    # the very last group splits into two 2-chain sub-units:
                    # the final DMA waits on only two tiny copies and the
                    # first sub-unit's DMA chain hides under the second's
                    av_sub(c, hw_, bl[0:2], [d0, d0 + 1], cps[0:2], nc.sync)
                    av_sub(c, hw_, bl[2:4], [d0 + 2, d0 + 3], cps[2:4],
                           nc.sync)
                else:
                    eng = nc.scalar if (c == 3 and grp == 0) else nc.sync
                    av_sub(c, hw_, bl, [d0, d0 + 1, d0 + 2, d0 + 3], cps,
                           eng)

            # chunk c+1's logits run between chunk c's two AV groups,
            # hiding the psum-bank WAR on group 0's copies (chunks 0/1
            # already emitted above).
            def den_unit(c, hw_, last):
                dnp = bank(1, f"den{c}", [1, 256])
                for cc in range(c + 1):
                    nc.tensor.matmul(
                        dnp[:, 0:hw_], lhsT=ones_sb[0:kp(cc)],
                        rhs=e_sb[cc][:, (c - cc) * 256:(c - cc) * 256 + hw_],
                        start=(cc == 0), stop=(cc == c))
                return dnp

            # ---- fused entry: Q chains, chunk-0/1 logits and chunk-0 AV
            # interleave so no psl-bank reuse ever waits on an exp and the
            # exp->tri latency hides under real matmuls.
            q_chain(0)
            # q1's chain with the first logits piece embedded mid-chain
            # (other-bank matmuls may interleave an accumulation): its
            # exp+tri then finish during the chain instead of after it
            q1_ps = bank(0, "psq1")
            for kc in range(KC):
                nc.tensor.matmul(
                    q1_ps[:], lhsT=wq_sb[:, kc, :],
                    rhs=xq_sb[:, kc, 512:1024],
                    start=(kc == 0), stop=(kc == KC - 1))
                if kc == 2:
                    logits_piece(0, 0, 256)
            dve_cp(qT_sb[:, 512:1024], q1_ps[:])
            q_chain(2)
            logits_piece(0, 256, 256)
            av_grp(0, 0, 256)
            logits_piece(0, 512, 512)
            dnp0 = den_unit(0, 256, False)
            logits_piece(1, 0, 512)
            dve_cp(den_sb[:, 0:256], dnp0[:, 0:256])
            nc.sync.dma_start(den[:, 0:256], den_sb[:, 0:256])
            logits_piece(1, 512, NQ - 768)
            logits_piece(0, 1024, NQ - 1024)
            av_grp(0, 1, 256)

            for ci, c in enumerate([1, 2, 4, 3]):
                hw_ = NQ - 1024 if c == 4 else 256
                last = ci == 3
                av_grp(c, 0, hw_)
                dnp = den_unit(c, hw_, last)
                for filler in {0: [2, 3], 1: [4]}.get(ci, []):
                    logits_chunk(filler)
                dve_cp(den_sb[:, c * 256:c * 256 + hw_], dnp[:, 0:hw_])
                nc.sync.dma_start(den[:, c * 256:c * 256 + hw_],
                                  den_sb[:, c * 256:c * 256 + hw_])
                av_grp(c, 1, hw_, final=last)

    nc.compile()
    return nc


def _get_nc():
    with _BUILD_LOCK:
        if "nc" not in _CACHE:
            _CACHE["nc"] = _build()
        return _CACHE["nc"]


def kernel(x, cross, Wq, Wk, Wv, mask):
    from concourse import bass_utils

    nc = _get_nc()
    bf = ml_dtypes.bfloat16

    x = np.asarray(x, np.float32)
    cross = np.asarray(cross, np.float32)
    scale = 1.0 / math.sqrt(DA)

    def pack_w(w):  # [DA, D] -> [P, KC*DA] in "p (kc m)" order
        wT = np.ascontiguousarray(np.asarray(w, np.float32).T)  # [D, DA]
        return np.ascontiguousarray(
            wT.reshape(KC, P, DA).transpose(1, 0, 2).reshape(P, KC * DA)
        ).astype(bf)

    wq_h = pack_w(np.asarray(Wq, np.float32) * scale)
    wk_h = pack_w(Wk)
    wv_h = np.ascontiguousarray(np.asarray(Wv, np.float32).T).astype(bf)
    ones_h = np.ones((P, 1), np.float32).astype(bf)

    in_maps = []
    meta = []
    for b in range(B):
        m = np.asarray(mask[b])
        U = np.where(m == 1)[0]
        order = np.concatenate([U, np.where(m == 0)[0]])
        n = len(U)
        assert n <= NQ, f"compact length {n} exceeds padded {NQ}"
        xqf = np.zeros((D, NQ), np.float32)
        xqf[:, :n] = x[b][U].T
        xq_h = xqf.astype(bf)
        for p in range(2):
            keys = order[p::2]
            na = len(U[p::2])
            assert 384 <= na <= NKL
            idx = np.arange(NCH * P).reshape(NCH, P).T  # [P, NCH]
            kbias_h = np.where(idx < na, 0.0, -BIG).astype(np.float32)
            jj = np.arange(256)[None, :]
            rr = np.arange(P)[:, None]
            tri_h = (2 * rr + p <= jj).astype(np.float32).astype(bf)
            xv_h = x[b][keys].T.astype(bf)
            in_maps.append({
                "xw": np.concatenate(
                    [xv_h[:, 0:128], wv_h[:, 0:512],
                     xv_h[:, 128:1024], wv_h[:, 512:1024]], axis=1),
                "xq": xq_h,
                "ck": np.ascontiguousarray(cross[b][keys[:NKL]].T).astype(bf),
                "wq": wq_h, "wk": wk_h,
                "kbias": kbias_h, "tri": tri_h, "ones": ones_h,
            })
            meta.append((b, p, n, order))

    _CACHE["in_maps"] = in_maps
    res = bass_utils.run_bass_kernel_spmd(
        nc, in_maps, core_ids=list(range(NCORES)))

    out = np.empty((B, S, D), np.float32)
    for b in range(B):
        n = meta[2 * b][2]
        order = meta[2 * b][3]
        U = order[:n]
        rA, rB = res.results[2 * b], res.results[2 * b + 1]
        avsum = rA["avT"][:, :n].astype(np.float32) \
            + rB["avT"][:, :n].astype(np.float32)
        densum = rA["den"][0, :n] + rB["den"][0, :n]
        out[b, U] = (avsum / densum[None, :]).T
        for p in range(2):
            r = res.results[2 * b + p]
            ranks = np.array([j for j in range(n, 2048) if j % 2 == p])
            ii = (ranks - p) // 2 - 384
            out[b, order[ranks]] = r["vout"][ii].astype(np.float32)
    return out
